# revision 59
# baseline (speedup 1.0000x reference)
"""FMoE forward (NaiveGate top-2, per-expert FFN, score-weighted combine) on 8 trn2 cores.

Strategy: hidden-dimension sharding (tensor-parallel within every expert).
The gate is computed on host as part of dispatch; the 8192 routed
(token, expert) slots are laid out expert-contiguously, and EVERY core
processes ALL slots but only a 512-wide slice of d_hidden per expert:

  core c:  H_c = gelu(X @ W1[e][:, c*512:(c+1)*512] + b1_slice)
           Y_c = H_c @ W2[e][c*512:(c+1)*512, :]        (partial sums)

The host sums the 8 partial Y arrays, applies the per-slot gate weight
(linear in Y, so it commutes with the hidden-slice sum), and scatter-adds
into the full [4096, 1024] output (+ w * b2 exactly).

Why this beats expert-parallel: per-core PE work is exactly
sum(C_e)/8 = 1024 token-slots regardless of routing imbalance (the
max-loaded expert no longer sets the makespan), the per-core weight
footprint stays 16 MB, and all 8 cores run the IDENTICAL program (true
SPMD) -- only the weight values differ.

Device kernel (per core, fp16 matmuls, fp32 PSUM accum):
  token slabs of <=512 slots, near-equal per expert, no 128-padding;
  mm1: lhsT = W1-slice chunk [128d, 128h], rhs = X^T slab [128d, S]
       -> H^T chunk [128h, S] PSUM (8 k-chunks); ScalarE gelu+b1 -> fp16.
  mm2: lhsT = W2-slice chunk [128h, 128d], rhs = H^T [128h, S]
       -> Y^T [128d, S] PSUM (4 h-chunks); ScalarE copy -> fp16, DMA out.
  mm2 of slab j is emitted after mm1 of slab j+1 (software pipeline) so
  the PE never waits on the gelu between mm1 and mm2 of the same slab.
"""

import os
import sys

import numpy as np

for _p in ("/opt/trn_rl_repo",):
    if _p not in sys.path and os.path.isdir(_p):
        sys.path.insert(0, _p)

N_TOKENS = 4096
D_MODEL = 1024
D_HIDDEN = 4096
N_EXPERT = 8
TOP_K = 2
P = 128
KO = D_MODEL // P  # 8 contraction chunks for mm1
NCORES = 8
HB = D_HIDDEN // NCORES  # 512-wide hidden slice per core
HSL = HB // P  # 4 hidden chunks per expert slice
DM = D_MODEL // P  # 8 output-partition chunks of Y^T
SMAX = 512  # max matmul moving dim (one PSUM bank)

# filled by kernel() for test harness introspection
last_results = None

_nc_cache = {}


def _near_equal(c4, cap=SMAX):
    """Split c4 (multiple of 4) into near-equal slabs <=cap, descending."""
    if c4 == 0:
        return []
    n = -(-c4 // cap)
    q = c4 // 4
    base, extra = divmod(q, n)
    return [4 * (base + 1)] * extra + [4 * base] * (n - extra)


def _expert_slabs(c4):
    return _near_equal(c4)


def _plan(loads):
    """Expert order + slab list. First expert's slabs ascend steeply
    (128, 384, ... -> compute starts while DMA bandwidth ramps); the last
    expert ends on [256, 128] so the un-overlappable tail is tiny."""
    c4s = [max(4, -(-c // 4) * 4) if c > 0 else 0 for c in loads]
    per_e = {e: _expert_slabs(c4s[e]) for e in range(N_EXPERT)}
    order = sorted(
        (e for e in range(N_EXPERT) if per_e[e]),
        key=lambda e: (len(per_e[e]), -c4s[e]),
    )
    first_e = order[0]
    if c4s[first_e] >= 1024:
        per_e[first_e] = [128, 256, 384] + sorted(_near_equal(c4s[first_e] - 768))
    last_e = order[-1]
    if c4s[last_e] >= 768:
        per_e[last_e] = _near_equal(c4s[last_e] - 128) + [128]
    slabs = []  # (expert, size, slot_offset)
    tok0 = 0
    for i, e in enumerate(order):
        for s in per_e[e]:
            slabs.append((e, s, tok0))
            tok0 += s
    offs = {}
    for e, s, t0 in slabs:
        offs.setdefault(e, t0)
    return order, slabs, offs, tok0, c4s


def _build_bass(struct):
    import concourse.mybir as mybir
    from concourse import bacc
    from concourse.tile import TileContext

    f16 = mybir.dt.float16
    f32 = mybir.dt.float32
    GELU = mybir.ActivationFunctionType.Gelu_apprx_tanh

    slabs = [(e, s, t0) for (e, s, t0) in struct]
    TOT = slabs[-1][2] + slabs[-1][1]

    nc = bacc.Bacc("TRN2", target_bir_lowering=False, debug=False, num_devices=NCORES)

    x_d = nc.declare_dram_parameter("x", [P, KO * TOT], f16, isOutput=False)
    w1_d = nc.declare_dram_parameter("w1", [N_EXPERT, P, HSL, KO * P], f16, isOutput=False)
    w2_d = nc.declare_dram_parameter("w2", [N_EXPERT, P, HSL, D_MODEL], f16, isOutput=False)
    # b1 padded to 128 f32/partition (512 B): a [P, 32] transfer has 128 B
    # partition runs -> descriptor-dominated, and it BLOCKS the scalar HWDGE
    # ring ~3us right when W1[e0] must move.
    b1_d = nc.declare_dram_parameter("b1", [P, 4 * N_EXPERT * HSL], f32, isOutput=False)
    out_d = nc.declare_dram_parameter("out", [P, DM * TOT], f16, isOutput=True)

    with TileContext(nc) as tc:
        with (
            tc.tile_pool(name="wpool", bufs=1) as wpool,
            tc.tile_pool(name="xpool", bufs=4) as xpool,
            tc.tile_pool(name="hpool", bufs=3) as hpool,
            tc.tile_pool(name="ypool", bufs=3) as ypool,
            tc.tile_pool(name="ps1", bufs=3, space="PSUM") as ps1,
            tc.tile_pool(name="ps2", bufs=5, space="PSUM") as ps2,
        ):
            w1_sb = wpool.tile([P, N_EXPERT * HSL, KO * P], f16)
            w2_sb = wpool.tile([P, N_EXPERT * HSL, D_MODEL], f16)
            b1_sb = wpool.tile([P, 4 * N_EXPERT * HSL], f32)

            # Startup DMAs with strict ring discipline. The 16 SDMA engines
            # round-robin across ALL queued transfers at packet granularity,
            # so a small critical transfer only completes fast if little else
            # is co-draining. Rings: sync = x only (in slab order, run-ahead
            # bounded by xpool bufs), scalar = all weights (run-ahead bounded
            # by the activations ahead of each prefetch in its FIFO queue),
            # gpsimd = y outputs only.
            # x and y tiles are FLAT [P, KO*S] / [P, DM*S]: the host already
            # lays each slab block contiguously, so flat tiles make every
            # x/y DMA a fully-contiguous 2D transfer (2-8 KB runs per
            # partition, line-rate descriptors). A 3D tile with SMAX row
            # stride would fragment the lead-in transfers into 256-512 B
            # descriptor runs at a fraction of line rate.
            # warm-up memset first in the gpsimd queue (SWDGE issue ops cost
            # ~1us of Q7 each; the memset must not sit behind them)
            warm = wpool.tile([P, 320], f16)
            nc.gpsimd.memset(warm, 0.0)

            pre_x = {}
            for j in range(min(2, len(slabs))):
                ej, Sj, tj = slabs[j]
                pre_x[j] = xpool.tile([P, KO * SMAX], f16, tag="x", name="x_sb")[
                    :, : KO * Sj
                ]
            # The critical first weights ride the SYNC ring ahead of x: a
            # ring is strict FIFO, which is the only real priority mechanism
            # (across rings the SDMA engines round-robin per packet, and x
            # packets are 4x bigger, so a "parallel" scalar-ring weight load
            # gets ~20% of the bandwidth and arrives late).
            e0 = slabs[0][0]
            eb = next((ee for ee, _, _ in slabs if ee != e0), None)
            w1e0 = w1_d[e0]
            # Keep the sync ring's early traffic to just ho0+x0+ho1+x1, in
            # need order: spreading these across rings posts descriptors
            # sooner but costs x0 its SDMA round-robin share (measured
            # worse). b1 on scalar, ho2/ho3 on gpsimd drain in parallel.
            nc.sync.dma_start(w1_sb[:, e0 * HSL, :], w1e0[:, 0, :])
            S0 = slabs[0][1]
            x0lo = KO * slabs[0][2]
            nc.sync.dma_start(pre_x[0][:, : KO * S0 // 2], x_d[:, x0lo : x0lo + KO * S0 // 2])
            nc.scalar.dma_start(b1_sb, b1_d[:, :])  # gates every gelu
            nc.scalar.dma_start(
                pre_x[0][:, KO * S0 // 2 :], x_d[:, x0lo + KO * S0 // 2 : x0lo + KO * S0]
            )
            nc.sync.dma_start(w1_sb[:, e0 * HSL + 1 : e0 * HSL + 2, :], w1e0[:, 1:2, :])
            nc.gpsimd.dma_start(w1_sb[:, e0 * HSL + 2 : e0 * HSL + 3, :], w1e0[:, 2:3, :])
            if 1 in pre_x:
                nc.sync.dma_start(
                    pre_x[1], x_d[:, KO * slabs[1][2] : KO * (slabs[1][2] + slabs[1][1])]
                )
            nc.gpsimd.dma_start(w1_sb[:, e0 * HSL + 3 : (e0 + 1) * HSL, :], w1e0[:, 3:, :])

            # PE warm-up: dependency-free matmuls keep the PE busy during the
            # DMA lead-in. Sized to the (short) critical DMA path: the PE
            # queue is in-order, so excess warm-up delays the first real mm.
            wps = ps1.tile([P, SMAX], mybir.dt.float32, tag="hps")
            for _ in range(16):
                nc.tensor.matmul(wps[:, :320], lhsT=warm[:, :P], rhs=warm, start=True, stop=True)

            w1_seen = {e0}
            w2_seen = {e0}
            pend = []  # slabs whose mm2 is pending, 2 deep: mm2(i) runs
            # between mm1(i+2) and mm1(i+3), so every mm1 has an mm2 queued
            # ahead of it in the in-order PE queue (hides a late x), and
            # each expert's W2 gets ~3 slabs of DMA lead time.

            for j, (e, S, t0) in enumerate(slabs):
                if j in pre_x:
                    x_sb = pre_x[j]
                else:
                    x_sb = xpool.tile([P, KO * SMAX], f16, tag="x", name="x_sb")[
                        :, : KO * S
                    ]
                    nc.sync.dma_start(x_sb, x_d[:, KO * t0 : KO * (t0 + S)])
                # Weight prefetch with two slabs of lead: W2[e'] one slab
                # ahead of the expert's first slab (its mm2 runs two further
                # slabs later), W1[e'] two slabs ahead of first use. All on
                # the scalar ring: its FIFO queue holds the previous slab's
                # activations ahead of these, which throttles issue to
                # compute progress (pure-DMA rings run arbitrarily far
                # ahead and steal SDMA bandwidth from critical transfers).
                # The Tile scheduler reorders each engine's stream by
                # dependency, so queue position alone cannot delay a
                # prefetch: an early-issued weight DMA steals SDMA
                # round-robin bandwidth from the critical lead-in loads.
                # Throttle each prefetch with a real dependency: a dummy
                # 1-element write into the target region sourced from the
                # previous slab's h tile (WAW with the DMA), so the DMA
                # issues only once compute has reached the previous slab.
                # dummies ride gpsimd (not vector/scalar: their strict FIFO
                # queues carry the mm2 copies, and a dummy blocked on a gelu
                # would stall ready copies behind it) and read the OLDEST
                # pending slab's h, whose gelu is already done by now.
                h_prev = pend[0][3] if pend else None

                def _throttled_w_load(w_sb, w_dram, en):
                    if h_prev is not None:
                        nc.gpsimd.tensor_scalar_mul(
                            w_sb[:, en * HSL, 0:1], h_prev[:, 0, 0:1], 1.0
                        )
                    nc.scalar.dma_start(
                        w_sb[:, en * HSL : (en + 1) * HSL, :], w_dram[en]
                    )

                if j == 1:
                    # W2 of the first expert, deferred out of the startup
                    # window (its mm2 only runs after mm1 of slab 2)
                    _throttled_w_load(w2_sb, w2_d, e0)
                if j + 1 < len(slabs):
                    en = slabs[j + 1][0]
                    if en not in w2_seen:
                        w2_seen.add(en)
                        _throttled_w_load(w2_sb, w2_d, en)
                if j + 2 < len(slabs):
                    en = slabs[j + 2][0]
                    if en not in w1_seen:
                        w1_seen.add(en)
                        _throttled_w_load(w1_sb, w1_d, en)

                if len(pend) == 2 and j >= 3:
                    _emit_mm2(
                        nc, ps2, ypool, w2_sb, out_d, pend.pop(0), mybir,
                        n_from_end=(len(slabs) - 1) - (j - 2),
                    )

                h_sb = hpool.tile([P, HSL, SMAX], f16, tag="h", name="h_sb")[:, :, :S]
                for ho in range(HSL):
                    hps = ps1.tile([P, SMAX], mybir.dt.float32, tag="hps", name="hps")[:, :S]
                    for k in range(KO):
                        nc.tensor.matmul(
                            hps,
                            lhsT=w1_sb[:, e * HSL + ho, k * P : (k + 1) * P],
                            rhs=x_sb[:, k * S : (k + 1) * S],
                            start=(k == 0),
                            stop=(k == KO - 1),
                        )
                    nc.scalar.activation(
                        h_sb[:, ho, :],
                        hps,
                        GELU,
                        bias=b1_sb[:, e * HSL + ho : e * HSL + ho + 1],
                    )

                if len(pend) == 2:  # only at j == 2: first mm2 after mm1(2)
                    _emit_mm2(
                        nc, ps2, ypool, w2_sb, out_d, pend.pop(0), mybir,
                        n_from_end=(len(slabs) - 1) - (j - 2),
                    )
                pend.append((e, S, t0, h_sb))

            _emit_mm2(nc, ps2, ypool, w2_sb, out_d, pend.pop(0), mybir, n_from_end=1)
            _emit_mm2(nc, ps2, ypool, w2_sb, out_d, pend.pop(0), mybir, n_from_end=0)

    nc.compile()
    return nc


def _emit_mm2(nc, ps2, ypool, w2_sb, out_d, prev, mybir, n_from_end):
    """mm2 for one slab: Y^T partial = W2_slice^T @ H^T, fp16 out + DMA.

    n_from_end: how many slabs still follow this one. For the last few
    slabs the output is streamed in halves (then chunks on the final slab)
    across both the gpsimd and sync rings -- x loads are finished by then,
    so sync is free, and nothing is left to hide a late bulk transfer
    behind."""
    f16 = mybir.dt.float16
    e, S, t0, h_sb = prev
    last = n_from_end == 0
    y_sb = ypool.tile([P, DM * SMAX], f16, tag="y", name="y_sb")[:, : DM * S]
    for m in range(DM):
        yps = ps2.tile([P, SMAX], mybir.dt.float32, tag="yps", name="yps")[:, :S]
        for ho in range(HSL):
            nc.tensor.matmul(
                yps,
                lhsT=w2_sb[:, e * HSL + ho, m * P : (m + 1) * P],
                rhs=h_sb[:, ho, :S],
                start=(ho == 0),
                stop=(ho == HSL - 1),
            )
        # PSUM f32 -> SBUF f16 cast, alternated across vector and scalar so
        # neither engine builds a backlog. On the final slab the very last
        # chunk is cast on vector: its queue is empty after c6, so the copy
        # starts the instant the last matmul drains (the scalar queue still
        # holds c5 + an issue op), shortening the last-byte chain.
        if m % 2 == 0 or (last and m == 7):
            nc.vector.tensor_scalar_mul(y_sb[:, m * S : (m + 1) * S], yps, 1.0)
        else:
            nc.scalar.copy(y_sb[:, m * S : (m + 1) * S], yps)
        if last:
            # The kernel ends ~3.5us after the LAST byte of data moves, so
            # the whole game is the final chunk's chain: keep the sync ring
            # EMPTY until m=7 (its issue+transfer then follow the last copy
            # immediately), keep the scalar queue free of issue ops before
            # the m=7 copy, and drain the earlier chunks as pairs on
            # gpsimd (+one scalar pair) while matmuls still run.
            if m in (1, 3, 5, 7):
                eng = (nc.gpsimd, nc.gpsimd, nc.scalar, nc.sync)[m // 2]
                eng.dma_start(
                    out_d[:, DM * t0 + (m - 1) * S : DM * t0 + (m + 1) * S],
                    y_sb[:, (m - 1) * S : (m + 1) * S],
                )
        elif not last and n_from_end <= 2 and m % 2 == 1:
            # near the end: stream quarters on gpsimd/scalar only -- the
            # sync ring must be idle when the final slab's last chunk posts
            eng = (nc.gpsimd, nc.scalar, nc.gpsimd, nc.scalar)[m // 2]
            eng.dma_start(
                out_d[:, DM * t0 + (m - 1) * S : DM * t0 + (m + 1) * S],
                y_sb[:, (m - 1) * S : (m + 1) * S],
            )
        elif not last and 2 < n_from_end <= 4 and m in (3, 7):
            eng = nc.gpsimd if m == 3 else nc.sync
            eng.dma_start(
                out_d[:, DM * t0 + (m - 3) * S : DM * t0 + (m + 1) * S],
                y_sb[:, (m - 3) * S : (m + 1) * S],
            )
    if not last and n_from_end > 4:
        nc.gpsimd.dma_start(out_d[:, DM * t0 : DM * (t0 + S)], y_sb)


def _route(moe_inp, Wg, bg):
    """Host gate: replicates NaiveGate (linear logits, top-2, softmax over the
    selected logits). Returns per-expert (token_idx, combine_weight)."""
    logits = moe_inp.astype(np.float32) @ Wg.astype(np.float32) + bg.astype(np.float32)
    order = np.argsort(-logits, axis=1, kind="stable")  # ties -> lower index first
    top_idx = order[:, :TOP_K]
    top_val = np.take_along_axis(logits, top_idx, axis=1)
    m = top_val.max(axis=1, keepdims=True)
    e = np.exp(top_val - m)
    gate = (e / e.sum(axis=1, keepdims=True)).astype(np.float32)
    toks, weights = [], []
    for ex in range(N_EXPERT):
        mask = top_idx == ex  # [N, K]; each token matches at most one slot
        t = np.nonzero(mask.any(axis=1))[0]
        w = gate[mask]  # row-major -> ascending token order, matches t
        toks.append(t)
        weights.append(w)
    return toks, weights


def kernel(**inputs):
    global last_results
    from concourse.bass_utils import run_bass_kernel_spmd

    moe_inp = np.asarray(inputs["moe_inp"], dtype=np.float32)
    Wg = np.asarray(inputs["Wg"], dtype=np.float32)
    bg = np.asarray(inputs["bg"], dtype=np.float32)
    W1 = np.asarray(inputs["W1"], dtype=np.float32)
    b1 = np.asarray(inputs["b1"], dtype=np.float32)
    W2 = np.asarray(inputs["W2"], dtype=np.float32)
    b2 = np.asarray(inputs["b2"], dtype=np.float32)

    toks, weights = _route(moe_inp, Wg, bg)
    loads = [len(t) for t in toks]
    order, slabs, offs, TOT, c4s = _plan(loads)

    key = tuple((e, s) for e, s, _ in slabs)
    if key not in _nc_cache:
        _nc_cache[key] = _build_bass(slabs)
    nc = _nc_cache[key]

    # Slot order: expert-contiguous per _plan's expert order; padded slots
    # (to multiples of 4) point at token 0 and are dropped on combine.
    slot_tok = np.zeros(TOT, dtype=np.int64)
    for e in order:
        slot_tok[offs[e] : offs[e] + loads[e]] = toks[e]

    X16 = moe_inp.astype(np.float16)
    xT = np.ascontiguousarray(X16[slot_tok].T)  # [D_MODEL, TOT]
    xblocks = []
    for e, S, t0 in slabs:
        xblocks.append(
            xT[:, t0 : t0 + S].reshape(KO, P, S).transpose(1, 0, 2).reshape(P, KO * S)
        )
    x_arr = np.ascontiguousarray(np.concatenate(xblocks, axis=1))

    W1_16 = W1.astype(np.float16)  # [E, 1024, 4096]
    W2_16 = W2.astype(np.float16)  # [E, 4096, 1024]
    in_maps = []
    for c in range(NCORES):
        sl = slice(c * HB, (c + 1) * HB)
        w1_arr = np.empty((N_EXPERT, P, HSL, KO * P), dtype=np.float16)
        w2_arr = np.empty((N_EXPERT, P, HSL, D_MODEL), dtype=np.float16)
        b1_arr = np.zeros((P, 4 * N_EXPERT * HSL), dtype=np.float32)
        for e in range(N_EXPERT):
            w1_arr[e] = (
                W1_16[e][:, sl]
                .reshape(KO, P, HSL, P)
                .transpose(1, 2, 0, 3)
                .reshape(P, HSL, KO * P)
            )
            w2_arr[e] = W2_16[e][sl, :].reshape(HSL, P, D_MODEL).transpose(1, 0, 2)
            b1_arr[:, e * HSL : (e + 1) * HSL] = b1[e][sl].reshape(HSL, P).T
        in_maps.append({"x": x_arr, "w1": w1_arr, "w2": w2_arr, "b1": b1_arr})

    last_results = run_bass_kernel_spmd(nc, in_maps, core_ids=list(range(NCORES)))

    # Sum the per-core hidden-slice partials, decode slab layout, apply the
    # gate weights (linear in Y), scatter-add into the full output.
    ysum = last_results.results[0]["out"].astype(np.float32)
    for r in last_results.results[1:]:
        ysum += r["out"]
    Yfull = np.empty((TOT, D_MODEL), dtype=np.float32)
    for e, S, t0 in slabs:
        blk = ysum[:, DM * t0 : DM * (t0 + S)].reshape(P, DM, S)
        Yfull[t0 : t0 + S] = blk.transpose(2, 1, 0).reshape(S, D_MODEL)

    out = np.zeros((N_TOKENS, D_MODEL), dtype=np.float32)
    for e in order:
        w = weights[e]
        seg = Yfull[offs[e] : offs[e] + loads[e]]
        out[toks[e]] += w[:, None] * seg + w[:, None] * b2[e][None, :]
    return out


if __name__ == "__main__":
    rng = np.random.default_rng(0)
    demo = {
        "moe_inp": rng.standard_normal((N_TOKENS, D_MODEL), dtype=np.float32),
        "attn_weights": rng.random((4, N_TOKENS, N_TOKENS), dtype=np.float32),
        "Wg": rng.standard_normal((D_MODEL, N_EXPERT), dtype=np.float32) / 32,
        "bg": np.zeros((N_EXPERT,), np.float32),
        "W1": rng.standard_normal((N_EXPERT, D_MODEL, D_HIDDEN), dtype=np.float32) / 32,
        "b1": np.zeros((N_EXPERT, D_HIDDEN), np.float32),
        "W2": rng.standard_normal((N_EXPERT, D_HIDDEN, D_MODEL), dtype=np.float32) / 64,
        "b2": np.zeros((N_EXPERT, D_MODEL), np.float32),
    }
    o = kernel(**demo)
    print(o.shape, o.dtype)

